# revision 45
# baseline (speedup 1.0000x reference)
"""Trainium2 Bass kernel for a 4-layer dense transformer block (nn_Block_spe).

Sharding: 8 cores = 2 groups of 4 (one group per batch element). Within a
group each core owns 512 query rows (sequence split). Per layer each core
projects q/k/v for its own rows, one AllGather (bf16, groups of 4) shares
k/v across the group, attention + MLP run fully local to the core's rows.
Residual stream stays in fp32 on-chip across all 4 layers; matmuls in bf16.

Everything on-chip uses the transposed [feature, token] layout so matmul
lhsT operands are the natural weight layouts and rhs is always [128, 512].
"""

import numpy as np
import ml_dtypes

import concourse.bass as bass
import concourse.mybir as mybir
import concourse.tile as tile
from concourse import bacc, bass_utils
from concourse.masks import make_identity

dt = mybir.dt
AF = mybir.ActivationFunctionType
BF16 = ml_dtypes.bfloat16

B, S, D = 2, 2048, 1024
H, HD = 16, 64
FF = 4096
DEPTH = 4
N_CORES = 8
GROUP = 4
S_LOC = S // GROUP          # 512 tokens per core
TOK = S_LOC                 # free dim of every matmul
MT = D // 128               # 8 feature tiles
KT = S // 128               # 16 key tiles (global)
FT = FF // 128              # 32 ff tiles
TT = S_LOC // 128           # 4 local token tiles
REPLICA_GROUPS = [[0, 1, 2, 3], [4, 5, 6, 7]]
SCALE = float(HD) ** -0.5
NEG = -1e30


def build_nc(s_loc=S_LOC, depth=DEPTH, ff=FF, n_cores=N_CORES, group=GROUP,
             kt_eff=None, sim_safe=False, local_collective=False):
    """Build the per-core SPMD program. Parametrized so a scaled-down
    version can run under CoreSim (sim_safe swaps Gelu for Identity since
    CoreSim lacks a Gelu implementation). local_collective replaces the
    AllGather with equivalent local DMA traffic so the single-core
    TimelineSim cost model can time the kernel.

    kt_eff: number of 128-key tiles after masked-key compaction (masked
    keys contribute exactly zero attention, so k/v rows are gathered by a
    host-provided index list; padding slots carry a -1e30 exp bias)."""
    act_mlp = AF.Identity if sim_safe else AF.Gelu
    tok = s_loc
    tt = s_loc // 128
    kt_n = group * tt
    if kt_eff is None:
        kt_eff = kt_n
    keysp = kt_eff * 128
    ft = ff // 128
    groups = [list(range(g * group, (g + 1) * group)) for g in range(n_cores // group)]

    nc = bacc.Bacc("TRN2", num_devices=n_cores, debug=False)

    # ---- DRAM I/O (host pre-tiles everything; see kernel() below) ----
    xT_d = nc.dram_tensor("xT", [128, MT, tok], dt.float32, kind="ExternalInput")
    mask_d = nc.dram_tensor("maskb", [128, kt_eff], dt.float32, kind="ExternalInput")
    wq_d = nc.dram_tensor("wq", [depth, MT, 128, MT, 128], dt.bfloat16, kind="ExternalInput")
    wk_d = nc.dram_tensor("wk", [depth, MT, 128, MT, 128], dt.bfloat16, kind="ExternalInput")
    wv_d = nc.dram_tensor("wv", [depth, 4, 128, MT, 256], dt.bfloat16, kind="ExternalInput")
    w1_d = nc.dram_tensor("w1", [depth, ft, 128, MT, 128], dt.bfloat16, kind="ExternalInput")
    w2_d = nc.dram_tensor("w2", [depth, MT, 128, ft, 128], dt.bfloat16, kind="ExternalInput")
    bq_d = nc.dram_tensor("bq", [depth, 128, MT], dt.float32, kind="ExternalInput")
    bk_d = nc.dram_tensor("bk", [depth, 128, MT], dt.float32, kind="ExternalInput")
    bv_d = nc.dram_tensor("bv", [depth, 128, D], dt.float32, kind="ExternalInput")
    b1_d = nc.dram_tensor("b1", [depth, 128, ft], dt.float32, kind="ExternalInput")
    b2_d = nc.dram_tensor("b2", [depth, 128, MT], dt.float32, kind="ExternalInput")
    y_d = nc.dram_tensor("yT", [128, MT, tok], dt.float32, kind="ExternalOutput")

    kblk = D * tok            # bf16 elems per rank in the k half of the AG
    vblk = s_loc * D          # bf16 elems per rank in the v half

    with tile.TileContext(nc) as tc:
        with (
            tc.tile_pool(name="per", bufs=1) as per,     # persistent state
            tc.tile_pool(name="sb", bufs=2) as sb,       # staging / per-layer
            tc.tile_pool(name="sb1", bufs=1) as sb1,     # k/v staging (single)
            tc.tile_pool(name="se", bufs=3) as se,       # expT pipeline
            tc.tile_pool(name="wp", bufs=3) as wp,       # small weight chunks
            tc.tile_pool(name="wp2", bufs=2) as wp2,     # big weight chunks
            tc.tile_pool(name="ps", bufs=2, space="PSUM") as ps,
            tc.tile_pool(name="ps2", bufs=2, space="PSUM") as ps2,
            tc.tile_pool(name="po_p", bufs=1, space="PSUM") as po_p,
            tc.tile_pool(name="dram", bufs=2, space="DRAM") as dram,
        ):
            # persistent tiles
            xT = per.tile([128, MT, tok], dt.float32)
            xT_bf = per.tile([128, MT, tok], dt.bfloat16)
            qT = per.tile([128, MT, tok], dt.bfloat16)
            kT_full = per.tile([128, MT, keysp], dt.bfloat16)
            V_int = per.tile([128, kt_eff, H, HD + 1], dt.bfloat16)
            oT = per.tile([128, MT, tok], dt.bfloat16)
            hT = per.tile([128, ft, tok], dt.bfloat16)
            maskb = per.tile([128, kt_eff], dt.float32)
            ident = per.tile([128, 128], dt.bfloat16)
            bv_bc = per.tile([128, D], dt.float32)

            make_identity(nc, ident[:])
            nc.sync.dma_start(xT[:], xT_d.ap())
            nc.sync.dma_start(maskb[:], mask_d.ap())
            nc.gpsimd.memset(V_int[:, :, :, HD], 1.0)

            for l in range(depth):
                # ---- biases for this layer ----
                bq_t = sb.tile([128, MT], dt.float32, tag="bq")
                b1_t = sb.tile([128, ft], dt.float32, tag="b1")
                b2_t = sb.tile([128, MT], dt.float32, tag="b2")
                nc.sync.dma_start(bq_t[:], bq_d.ap()[l])
                nc.sync.dma_start(b1_t[:], b1_d.ap()[l])
                nc.sync.dma_start(b2_t[:], b2_d.ap()[l])
                nc.sync.dma_start(bv_bc[:], bv_d.ap()[l])
                bk_t = sb.tile([128, MT], dt.float32, tag="bk")
                nc.sync.dma_start(bk_t[:], bk_d.ap()[l])

                # ---- downcast residual for matmul inputs (per m-tile so the
                # next layer's projections can overlap this layer's MLP tail)
                for m in range(MT):
                    nc.vector.tensor_copy(xT_bf[:, m, :], xT[:, m, :])

                agin_k = dram.tile([kblk], dt.bfloat16, tag="agin_k")
                agin_v = dram.tile([vblk], dt.bfloat16, tag="agin_v")

                # ---- k projection (transposed feature-major layout) ----
                kst = sb1.tile([128, MT, tok], dt.bfloat16, tag="kstage")
                for m in range(MT):
                    wc = wp.tile([128, MT, 128], dt.bfloat16, tag="wqk")
                    nc.sync.dma_start(wc[:], wk_d.ap()[l, m])
                    pk = ps.tile([128, tok], dt.float32, tag="pp")
                    for k in range(MT):
                        nc.tensor.matmul(pk[:], lhsT=wc[:, k, :], rhs=xT_bf[:, k, :],
                                         start=(k == 0), stop=(k == MT - 1))
                    nc.vector.tensor_scalar_add(kst[:, m, :], pk[:],
                                                bk_t[:, m : m + 1])
                nc.sync.dma_start(
                    agin_k.opt().rearrange("(m p t) -> p m t", p=128, t=tok),
                    kst[:],
                )
                agout_k = dram.tile([group * kblk], dt.bfloat16, tag="agout_k")
                if local_collective:
                    for r in range(group):
                        nc.sync.dma_start(
                            agout_k.opt()[r * kblk : (r + 1) * kblk], agin_k.opt())
                else:
                    nc.gpsimd.collective_compute(
                        "AllGather", mybir.AluOpType.bypass,
                        ins=[agin_k.opt()], outs=[agout_k.opt()],
                        replica_groups=groups,
                    )

                # ---- v projection (natural layout) ----
                vst = sb1.tile([128, tt, D], dt.bfloat16, tag="vstage")
                for cc in range(4):
                    wc = wp2.tile([128, MT, 256], dt.bfloat16, tag="wv")
                    nc.sync.dma_start(wc[:], wv_d.ap()[l, cc])
                    for t in range(tt):
                        pv = ps.tile([128, 256], dt.float32, tag="pp")
                        for k in range(MT):
                            nc.tensor.matmul(
                                pv[:], lhsT=xT_bf[:, k, t * 128 : (t + 1) * 128],
                                rhs=wc[:, k, :],
                                start=(k == 0), stop=(k == MT - 1))
                        nc.vector.tensor_add(
                            vst[:, t, cc * 256 : (cc + 1) * 256], pv[:],
                            bv_bc[:, cc * 256 : (cc + 1) * 256])
                nc.sync.dma_start(
                    agin_v.opt().rearrange("(t p c) -> p t c", p=128, c=D),
                    vst[:],
                )
                agout_v = dram.tile([group * vblk], dt.bfloat16, tag="agout_v")
                if local_collective:
                    for r in range(group):
                        nc.sync.dma_start(
                            agout_v.opt()[r * vblk : (r + 1) * vblk], agin_v.opt())
                else:
                    nc.gpsimd.collective_compute(
                        "AllGather", mybir.AluOpType.bypass,
                        ins=[agin_v.opt()], outs=[agout_v.opt()],
                        replica_groups=groups,
                    )

                # ---- q projection (overlaps the AllGathers) ----
                for m in range(MT):
                    wc = wp.tile([128, MT, 128], dt.bfloat16, tag="wqk")
                    nc.sync.dma_start(wc[:], wq_d.ap()[l, m])
                    pq = ps.tile([128, tok], dt.float32, tag="pp")
                    for k in range(MT):
                        nc.tensor.matmul(pq[:], lhsT=wc[:, k, :], rhs=xT_bf[:, k, :],
                                         start=(k == 0), stop=(k == MT - 1))
                    nc.vector.tensor_scalar_add(qT[:, m, :], pq[:],
                                                bq_t[:, m : m + 1])

                # ---- gather unmasked k/v rows out of the AllGather output ----
                # k rows transposed on the fly into feature-major [128, MT, keysp]
                # tokens are host-permuted so unmasked keys are the prefix:
                # only the first kt_eff tiles of the gathered k/v are loaded
                kt_done = 0
                for r in range(group):
                    cols = min(tok, keysp - r * tok)
                    if cols <= 0:
                        break
                    nc.sync.dma_start(
                        kT_full[:, :, r * tok : r * tok + cols],
                        agout_k.opt()[r * kblk : (r + 1) * kblk].rearrange(
                            "(m p t) -> p m t", p=128, t=tok)[:, :, 0:cols],
                    )
                    for t in range(cols // 128):
                        vofs = r * vblk + t * 128 * D
                        nc.sync.dma_start(
                            V_int[:, kt_done, :, 0:HD],
                            agout_v.opt()[vofs : vofs + 128 * D].rearrange(
                                "(p h d) -> p h d", p=128, h=H),
                        )
                        kt_done += 1

                # ---- attention, two heads interleaved (hides the
                # scores->exp->AV round-trip latency behind the other head's
                # matmuls) ----
                for hp in range(H // 2):
                    po0 = po_p.tile([65, tok], dt.float32, tag="po0")
                    po1 = po_p.tile([65, tok], dt.float32, tag="po1")
                    for kti in range(kt_eff):
                        pse = ps2.tile([128, 2 * tok], dt.float32, tag="pp2")
                        for rem in range(2):
                            nc.tensor.matmul(
                                pse[:, rem * tok : (rem + 1) * tok],
                                lhsT=kT_full[64 * rem : 64 * rem + 64, hp,
                                             kti * 128 : (kti + 1) * 128],
                                rhs=qT[64 * rem : 64 * rem + 64, hp, :],
                                start=True, stop=True)
                        # one double-width exp: both heads of the pair share
                        # the same key-tile mask column
                        et = se.tile([128, 2 * tok], dt.bfloat16, tag="expT")
                        nc.scalar.activation(et[:], pse[:], AF.Exp,
                                             bias=maskb[:, kti : kti + 1],
                                             scale=SCALE)
                        for rem, po in enumerate((po0, po1)):
                            nc.tensor.matmul(po[:], lhsT=V_int[:, kti, 2 * hp + rem, :],
                                             rhs=et[:, rem * tok : (rem + 1) * tok],
                                             start=(kti == 0), stop=(kti == kt_eff - 1))
                    for rem, po in enumerate((po0, po1)):
                        rec = sb.tile([1, tok], dt.float32, tag="rec")
                        nc.vector.reciprocal(rec[:], po[64:65, :])
                        bc = sb.tile([64, tok], dt.float32, tag="bc")
                        nc.gpsimd.partition_broadcast(bc[:], rec[:])
                        nc.vector.tensor_mul(
                            oT[64 * rem : 64 * rem + 64, hp, :], po[0:64, :], bc[:])

                # ---- MLP up (gelu), ff-tile pairs share a 2-bank psum ----
                for f in range(0, ft, 2):
                    ph = ps2.tile([128, 2 * tok], dt.float32, tag="pp2")
                    for fo in range(2):
                        wc = wp.tile([128, MT, 128], dt.bfloat16, tag="w1")
                        nc.sync.dma_start(wc[:], w1_d.ap()[l, f + fo])
                        for k in range(MT):
                            nc.tensor.matmul(ph[:, fo * tok : (fo + 1) * tok],
                                             lhsT=wc[:, k, :], rhs=oT[:, k, :],
                                             start=(k == 0), stop=(k == MT - 1))
                    for fo in range(2):
                        nc.scalar.activation(hT[:, f + fo, :],
                                             ph[:, fo * tok : (fo + 1) * tok],
                                             act_mlp,
                                             bias=b1_t[:, f + fo : f + fo + 1],
                                             scale=1.0)

                # ---- MLP down + o + residual ----
                hft = ft // 2
                for m in range(MT):
                    pm = ps.tile([128, tok], dt.float32, tag="pp")
                    for hc in range(2):
                        wc = wp2.tile([128, hft, 128], dt.bfloat16, tag="w2")
                        nc.sync.dma_start(wc[:], w2_d.ap()[l, m, :, hc * hft : (hc + 1) * hft, :])
                        for k in range(hft):
                            nc.tensor.matmul(pm[:], lhsT=wc[:, k, :],
                                             rhs=hT[:, hc * hft + k, :],
                                             start=(hc == 0 and k == 0), stop=False)
                    nc.tensor.matmul(pm[:], lhsT=ident[:], rhs=oT[:, m, :],
                                     start=False, stop=True)
                    tmp = sb.tile([128, tok], dt.float32, tag="tmp")
                    nc.vector.tensor_scalar_add(tmp[:], pm[:], b2_t[:, m : m + 1])
                    nc.vector.tensor_add(xT[:, m, :], xT[:, m, :], tmp[:])

            nc.sync.dma_start(y_d.ap(), xT[:])

    nc.compile()
    return nc


def _prep_inputs(x, mask, Wq, bq, Wk, bk, Wv, bv, W1, b1, W2, b2,
                 s_loc=S_LOC, depth=DEPTH, ff=FF, n_cores=N_CORES, group=GROUP):
    """Host-side shard + pre-tile. Tokens of each batch are permuted so
    unmasked tokens form a prefix: attention inside the block is
    permutation-equivariant (no positional encoding), so only the prefix
    ever needs to participate as keys. Returns (in_maps, kt_eff, perms)."""
    tok = s_loc
    ft = ff // 128
    s = group * s_loc

    def tile_w_cols(w, ncol):
        # [depth, D, Dout] -> [depth, Dout//ncol, 128, MT, ncol] bf16
        d_out = w.shape[2]
        r = w.reshape(depth, MT, 128, d_out // ncol, ncol)
        return np.ascontiguousarray(r.transpose(0, 3, 2, 1, 4)).astype(BF16)

    def tile_w2(w):
        # [depth, ff, D] -> [depth, MT, 128, ft, 128]
        r = w.reshape(depth, ft, 128, MT, 128)
        return np.ascontiguousarray(r.transpose(0, 3, 2, 1, 4)).astype(BF16)

    def tile_bias(b, nt):
        # [depth, X] -> [depth, 128, X//128] partition-major
        return np.ascontiguousarray(
            b.reshape(depth, nt, 128).transpose(0, 2, 1)).astype(np.float32)

    def bcast_bias(b):
        return np.ascontiguousarray(
            np.broadcast_to(b.reshape(depth, 1, D), (depth, 128, D))
        ).astype(np.float32)

    # masked keys are dead weight: permute unmasked tokens to the front,
    # so the live keys are a contiguous prefix of the gathered k/v
    perms = []
    for b_idx in range(B):
        live = np.nonzero(mask[b_idx, :s] != 0)[0]
        dead = np.nonzero(mask[b_idx, :s] == 0)[0]
        perms.append(np.concatenate([live, dead]))
    n_live = [int((mask[b_idx, :s] != 0).sum()) for b_idx in range(B)]
    kt_eff = max(1, max((u + 127) // 128 for u in n_live))
    keysp = kt_eff * 128

    shared = {
        "wq": tile_w_cols(Wq, 128),
        "wk": tile_w_cols(Wk, 128),
        "wv": tile_w_cols(Wv, 256),
        "w1": tile_w_cols(W1, 128),
        "w2": tile_w2(W2),
        "bq": tile_bias(bq, MT),
        "bk": tile_bias(bk, MT),
        "b1": tile_bias(b1, ft),
        "b2": tile_bias(b2, MT),
        "bv": bcast_bias(bv),
    }
    in_maps = []
    for c in range(n_cores):
        b_idx, r_idx = divmod(c, group)
        xp = x[b_idx][perms[b_idx]]                           # [s, D] permuted
        xl = xp[r_idx * s_loc : (r_idx + 1) * s_loc, :]       # [s_loc, D]
        xT = np.ascontiguousarray(
            xl.T.reshape(MT, 128, tok).transpose(1, 0, 2)).astype(np.float32)
        u = n_live[b_idx]
        mb = np.full(keysp, NEG, np.float32)
        mb[:u] = 0.0
        mb = np.ascontiguousarray(mb.reshape(kt_eff, 128).T)
        in_maps.append({"xT": xT, "maskb": mb, **shared})
    return in_maps, kt_eff, perms


def _assemble(results, perms, s_loc=S_LOC, n_cores=N_CORES, group=GROUP):
    s = group * s_loc
    out = np.empty((B, s, D), dtype=np.float32)
    for c in range(n_cores):
        b_idx, r_idx = divmod(c, group)
        yT = results[c]["yT"]  # [128, MT, tok]
        xl = yT.transpose(1, 0, 2).reshape(D, s_loc).T
        out[b_idx, perms[b_idx][r_idx * s_loc : (r_idx + 1) * s_loc]] = xl
    return out


_NC_CACHE = {}


def run(inputs, trace=False):
    in_maps, kt_eff, perms = _prep_inputs(**inputs)
    if kt_eff not in _NC_CACHE:
        _NC_CACHE[kt_eff] = build_nc(kt_eff=kt_eff)
    nc = _NC_CACHE[kt_eff]
    res = bass_utils.run_bass_kernel_spmd(
        nc, in_maps, core_ids=list(range(N_CORES)), trace=trace)
    return _assemble(res.results, perms), res


def kernel(**inputs):
    inputs = {k: np.asarray(v) for k, v in inputs.items()}
    out, _ = run(inputs)
    return out


# revision 47
# speedup vs baseline: 1.0449x; 1.0449x over previous
"""Trainium2 Bass kernel for a 4-layer dense transformer block (nn_Block_spe).

Sharding: 8 cores = 2 groups of 4 (one group per batch element). Within a
group each core owns 512 query rows (sequence split). Per layer each core
projects q/k/v for its own rows, two AllGathers (bf16, groups of 4) share
k/v across the group, attention + MLP run fully local to the core's rows.
Residual stream stays in fp32 on-chip across all 4 layers; matmuls in bf16.

Masked-key compaction: the host permutes each batch's tokens so unmasked
tokens form a prefix (the block has no positional encoding, so attention
is permutation-equivariant; outputs are un-permuted at the end). Only the
first ceil(unmasked/128) key tiles are loaded and attended over — masked
keys would contribute exactly zero attention weight anyway. Padding slots
in the last tile are killed by a -1e30 bias fused into the exp.

Everything on-chip uses the transposed [feature, token] layout so matmul
lhsT operands are the natural weight layouts and rhs is always [128, 512].
"""

import numpy as np
import ml_dtypes

import concourse.bass as bass
import concourse.mybir as mybir
import concourse.tile as tile
from concourse import bacc, bass_utils
from concourse.masks import make_identity

dt = mybir.dt
AF = mybir.ActivationFunctionType
BF16 = ml_dtypes.bfloat16

B, S, D = 2, 2048, 1024
H, HD = 16, 64
FF = 4096
DEPTH = 4
N_CORES = 8
GROUP = 4
S_LOC = S // GROUP          # 512 tokens per core
TOK = S_LOC                 # free dim of every matmul
MT = D // 128               # 8 feature tiles
KT = S // 128               # 16 key tiles (global)
FT = FF // 128              # 32 ff tiles
TT = S_LOC // 128           # 4 local token tiles
REPLICA_GROUPS = [[0, 1, 2, 3], [4, 5, 6, 7]]
SCALE = float(HD) ** -0.5
NEG = -1e30


def build_nc(s_loc=S_LOC, depth=DEPTH, ff=FF, n_cores=N_CORES, group=GROUP,
             kt_eff=None, sim_safe=False, local_collective=False):
    """Build the per-core SPMD program. Parametrized so a scaled-down
    version can run under CoreSim (sim_safe swaps Gelu for Identity since
    CoreSim lacks a Gelu implementation). local_collective replaces the
    AllGather with equivalent local DMA traffic so the single-core
    TimelineSim cost model can time the kernel.

    kt_eff: number of 128-key tiles after masked-key compaction (the host
    permutes unmasked tokens to the front, so live keys are a contiguous
    prefix of the AllGather output; padding slots carry a -1e30 exp bias)."""
    act_mlp = AF.Identity if sim_safe else AF.Gelu
    tok = s_loc
    tt = s_loc // 128
    kt_n = group * tt
    if kt_eff is None:
        kt_eff = kt_n
    keysp = kt_eff * 128
    ft = ff // 128
    groups = [list(range(g * group, (g + 1) * group)) for g in range(n_cores // group)]

    nc = bacc.Bacc("TRN2", num_devices=n_cores, debug=False)

    # ---- DRAM I/O (host pre-tiles everything; see kernel() below) ----
    xT_d = nc.dram_tensor("xT", [128, MT, tok], dt.float32, kind="ExternalInput")
    mask_d = nc.dram_tensor("maskb", [128, kt_eff], dt.float32, kind="ExternalInput")
    wq_d = nc.dram_tensor("wq", [depth, MT, 128, MT, 128], dt.bfloat16, kind="ExternalInput")
    wk_d = nc.dram_tensor("wk", [depth, MT, 128, MT, 128], dt.bfloat16, kind="ExternalInput")
    wv_d = nc.dram_tensor("wv", [depth, 4, 128, MT, 256], dt.bfloat16, kind="ExternalInput")
    w1_d = nc.dram_tensor("w1", [depth, ft, 128, MT, 128], dt.bfloat16, kind="ExternalInput")
    w2_d = nc.dram_tensor("w2", [depth, MT, 128, ft, 128], dt.bfloat16, kind="ExternalInput")
    bq_d = nc.dram_tensor("bq", [depth, 128, MT], dt.float32, kind="ExternalInput")
    bk_d = nc.dram_tensor("bk", [depth, 128, MT], dt.float32, kind="ExternalInput")
    bv_d = nc.dram_tensor("bv", [depth, 128, D], dt.float32, kind="ExternalInput")
    b1_d = nc.dram_tensor("b1", [depth, 128, ft], dt.float32, kind="ExternalInput")
    b2_d = nc.dram_tensor("b2", [depth, 128, MT], dt.float32, kind="ExternalInput")
    y_d = nc.dram_tensor("yT", [128, MT, tok], dt.float32, kind="ExternalOutput")

    kblk = D * tok            # bf16 elems per rank in the k half of the AG
    vblk = s_loc * D          # bf16 elems per rank in the v half

    with tile.TileContext(nc) as tc:
        with (
            tc.tile_pool(name="per", bufs=1) as per,     # persistent state
            tc.tile_pool(name="sb", bufs=2) as sb,       # staging / per-layer
            tc.tile_pool(name="sb1", bufs=1) as sb1,     # k/v staging (single)
            tc.tile_pool(name="se", bufs=3) as se,       # expT pipeline
            tc.tile_pool(name="wp", bufs=5) as wp,       # small weight chunks
            tc.tile_pool(name="wp2", bufs=3) as wp2,     # big weight chunks
            tc.tile_pool(name="ps", bufs=2, space="PSUM") as ps,
            tc.tile_pool(name="ps2", bufs=2, space="PSUM") as ps2,
            tc.tile_pool(name="po_p", bufs=1, space="PSUM") as po_p,
            tc.tile_pool(name="dram", bufs=2, space="DRAM") as dram,
        ):
            # persistent tiles
            xT = per.tile([128, MT, tok], dt.float32)
            xT_bf = per.tile([128, MT, tok], dt.bfloat16)
            qT = per.tile([128, MT, tok], dt.bfloat16)
            kT_full = per.tile([128, MT, keysp], dt.bfloat16)
            V_int = per.tile([128, kt_eff, H, HD + 1], dt.bfloat16)
            oT = per.tile([128, MT, tok], dt.bfloat16)
            hT = per.tile([128, ft, tok], dt.bfloat16)
            maskb = per.tile([128, kt_eff], dt.float32)
            ident = per.tile([128, 128], dt.bfloat16)
            bv_bc = per.tile([128, D], dt.float32)

            make_identity(nc, ident[:])
            nc.sync.dma_start(xT[:], xT_d.ap())
            nc.sync.dma_start(maskb[:], mask_d.ap())
            nc.gpsimd.memset(V_int[:, :, :, HD], 1.0)

            for l in range(depth):
                # ---- biases for this layer ----
                bq_t = sb.tile([128, MT], dt.float32, tag="bq")
                b1_t = sb.tile([128, ft], dt.float32, tag="b1")
                b2_t = sb.tile([128, MT], dt.float32, tag="b2")
                nc.sync.dma_start(bq_t[:], bq_d.ap()[l])
                nc.sync.dma_start(b1_t[:], b1_d.ap()[l])
                nc.sync.dma_start(b2_t[:], b2_d.ap()[l])
                nc.sync.dma_start(bv_bc[:], bv_d.ap()[l])
                bk_t = sb.tile([128, MT], dt.float32, tag="bk")
                nc.sync.dma_start(bk_t[:], bk_d.ap()[l])

                # ---- downcast residual for matmul inputs (per m-tile so the
                # next layer's projections can overlap this layer's MLP tail)
                for m in range(MT):
                    nc.vector.tensor_copy(xT_bf[:, m, :], xT[:, m, :])

                agin_k = dram.tile([kblk], dt.bfloat16, tag="agin_k")
                agin_v = dram.tile([vblk], dt.bfloat16, tag="agin_v")

                # ---- k projection (transposed feature-major layout) ----
                kst = sb1.tile([128, MT, tok], dt.bfloat16, tag="kstage")
                for m in range(MT):
                    wc = wp.tile([128, MT, 128], dt.bfloat16, tag="wqk")
                    nc.sync.dma_start(wc[:], wk_d.ap()[l, m])
                    pk = ps.tile([128, tok], dt.float32, tag="pp")
                    for k in range(MT):
                        nc.tensor.matmul(pk[:], lhsT=wc[:, k, :], rhs=xT_bf[:, k, :],
                                         start=(k == 0), stop=(k == MT - 1))
                    nc.vector.tensor_scalar_add(kst[:, m, :], pk[:],
                                                bk_t[:, m : m + 1])
                nc.sync.dma_start(
                    agin_k.opt().rearrange("(m p t) -> p m t", p=128, t=tok),
                    kst[:],
                )
                agout_k = dram.tile([group * kblk], dt.bfloat16, tag="agout_k")
                if local_collective:
                    for r in range(group):
                        nc.sync.dma_start(
                            agout_k.opt()[r * kblk : (r + 1) * kblk], agin_k.opt())
                else:
                    nc.gpsimd.collective_compute(
                        "AllGather", mybir.AluOpType.bypass,
                        ins=[agin_k.opt()], outs=[agout_k.opt()],
                        replica_groups=groups,
                    )

                # ---- v projection (natural layout) ----
                vst = sb1.tile([128, tt, D], dt.bfloat16, tag="vstage")
                for cc in range(4):
                    wc = wp2.tile([128, MT, 256], dt.bfloat16, tag="wv")
                    nc.sync.dma_start(wc[:], wv_d.ap()[l, cc])
                    for t in range(tt):
                        pv = ps.tile([128, 256], dt.float32, tag="pp")
                        for k in range(MT):
                            nc.tensor.matmul(
                                pv[:], lhsT=xT_bf[:, k, t * 128 : (t + 1) * 128],
                                rhs=wc[:, k, :],
                                start=(k == 0), stop=(k == MT - 1))
                        nc.vector.tensor_add(
                            vst[:, t, cc * 256 : (cc + 1) * 256], pv[:],
                            bv_bc[:, cc * 256 : (cc + 1) * 256])
                nc.sync.dma_start(
                    agin_v.opt().rearrange("(t p c) -> p t c", p=128, c=D),
                    vst[:],
                )
                agout_v = dram.tile([group * vblk], dt.bfloat16, tag="agout_v")
                if local_collective:
                    for r in range(group):
                        nc.sync.dma_start(
                            agout_v.opt()[r * vblk : (r + 1) * vblk], agin_v.opt())
                else:
                    nc.gpsimd.collective_compute(
                        "AllGather", mybir.AluOpType.bypass,
                        ins=[agin_v.opt()], outs=[agout_v.opt()],
                        replica_groups=groups,
                    )

                # ---- q projection (overlaps the AllGathers) ----
                for m in range(MT):
                    wc = wp.tile([128, MT, 128], dt.bfloat16, tag="wqk")
                    nc.sync.dma_start(wc[:], wq_d.ap()[l, m])
                    pq = ps.tile([128, tok], dt.float32, tag="pp")
                    for k in range(MT):
                        nc.tensor.matmul(pq[:], lhsT=wc[:, k, :], rhs=xT_bf[:, k, :],
                                         start=(k == 0), stop=(k == MT - 1))
                    nc.vector.tensor_scalar_add(qT[:, m, :], pq[:],
                                                bq_t[:, m : m + 1])

                # ---- gather unmasked k/v rows out of the AllGather output ----
                # k rows transposed on the fly into feature-major [128, MT, keysp]
                # tokens are host-permuted so unmasked keys are the prefix:
                # only the first kt_eff tiles of the gathered k/v are loaded
                kt_done = 0
                for r in range(group):
                    cols = min(tok, keysp - r * tok)
                    if cols <= 0:
                        break
                    nc.sync.dma_start(
                        kT_full[:, :, r * tok : r * tok + cols],
                        agout_k.opt()[r * kblk : (r + 1) * kblk].rearrange(
                            "(m p t) -> p m t", p=128, t=tok)[:, :, 0:cols],
                    )
                    for t in range(cols // 128):
                        vofs = r * vblk + t * 128 * D
                        nc.sync.dma_start(
                            V_int[:, kt_done, :, 0:HD],
                            agout_v.opt()[vofs : vofs + 128 * D].rearrange(
                                "(p h d) -> p h d", p=128, h=H),
                        )
                        kt_done += 1

                # ---- attention, two heads interleaved (hides the
                # scores->exp->AV round-trip latency behind the other head's
                # matmuls) ----
                for hp in range(H // 2):
                    po0 = po_p.tile([65, tok], dt.float32, tag="po0")
                    po1 = po_p.tile([65, tok], dt.float32, tag="po1")
                    for kti in range(kt_eff):
                        pse = ps2.tile([128, 2 * tok], dt.float32, tag="pp2")
                        for rem in range(2):
                            nc.tensor.matmul(
                                pse[:, rem * tok : (rem + 1) * tok],
                                lhsT=kT_full[64 * rem : 64 * rem + 64, hp,
                                             kti * 128 : (kti + 1) * 128],
                                rhs=qT[64 * rem : 64 * rem + 64, hp, :],
                                start=True, stop=True)
                        # one double-width exp: both heads of the pair share
                        # the same key-tile mask column
                        et = se.tile([128, 2 * tok], dt.bfloat16, tag="expT")
                        nc.scalar.activation(et[:], pse[:], AF.Exp,
                                             bias=maskb[:, kti : kti + 1],
                                             scale=SCALE)
                        for rem, po in enumerate((po0, po1)):
                            nc.tensor.matmul(po[:], lhsT=V_int[:, kti, 2 * hp + rem, :],
                                             rhs=et[:, rem * tok : (rem + 1) * tok],
                                             start=(kti == 0), stop=(kti == kt_eff - 1))
                    for rem, po in enumerate((po0, po1)):
                        rec = sb.tile([1, tok], dt.float32, tag="rec")
                        nc.vector.reciprocal(rec[:], po[64:65, :])
                        bc = sb.tile([64, tok], dt.float32, tag="bc")
                        nc.gpsimd.partition_broadcast(bc[:], rec[:])
                        nc.vector.tensor_mul(
                            oT[64 * rem : 64 * rem + 64, hp, :], po[0:64, :], bc[:])

                # ---- MLP up (gelu), ff-tile pairs share a 2-bank psum ----
                for f in range(0, ft, 2):
                    ph = ps2.tile([128, 2 * tok], dt.float32, tag="pp2")
                    for fo in range(2):
                        wc = wp.tile([128, MT, 128], dt.bfloat16, tag="w1")
                        nc.sync.dma_start(wc[:], w1_d.ap()[l, f + fo])
                        for k in range(MT):
                            nc.tensor.matmul(ph[:, fo * tok : (fo + 1) * tok],
                                             lhsT=wc[:, k, :], rhs=oT[:, k, :],
                                             start=(k == 0), stop=(k == MT - 1))
                    for fo in range(2):
                        nc.scalar.activation(hT[:, f + fo, :],
                                             ph[:, fo * tok : (fo + 1) * tok],
                                             act_mlp,
                                             bias=b1_t[:, f + fo : f + fo + 1],
                                             scale=1.0)

                # ---- MLP down + o + residual ----
                hft = ft // 2
                for m in range(MT):
                    pm = ps.tile([128, tok], dt.float32, tag="pp")
                    for hc in range(2):
                        wc = wp2.tile([128, hft, 128], dt.bfloat16, tag="w2")
                        nc.sync.dma_start(wc[:], w2_d.ap()[l, m, :, hc * hft : (hc + 1) * hft, :])
                        for k in range(hft):
                            nc.tensor.matmul(pm[:], lhsT=wc[:, k, :],
                                             rhs=hT[:, hc * hft + k, :],
                                             start=(hc == 0 and k == 0), stop=False)
                    nc.tensor.matmul(pm[:], lhsT=ident[:], rhs=oT[:, m, :],
                                     start=False, stop=True)
                    tmp = sb.tile([128, tok], dt.float32, tag="tmp")
                    nc.vector.tensor_scalar_add(tmp[:], pm[:], b2_t[:, m : m + 1])
                    nc.vector.tensor_add(xT[:, m, :], xT[:, m, :], tmp[:])

            nc.sync.dma_start(y_d.ap(), xT[:])

    nc.compile()
    return nc


def _prep_inputs(x, mask, Wq, bq, Wk, bk, Wv, bv, W1, b1, W2, b2,
                 s_loc=S_LOC, depth=DEPTH, ff=FF, n_cores=N_CORES, group=GROUP):
    """Host-side shard + pre-tile. Tokens of each batch are permuted so
    unmasked tokens form a prefix: attention inside the block is
    permutation-equivariant (no positional encoding), so only the prefix
    ever needs to participate as keys. Returns (in_maps, kt_eff, perms)."""
    tok = s_loc
    ft = ff // 128
    s = group * s_loc

    def tile_w_cols(w, ncol):
        # [depth, D, Dout] -> [depth, Dout//ncol, 128, MT, ncol] bf16
        d_out = w.shape[2]
        r = w.reshape(depth, MT, 128, d_out // ncol, ncol)
        return np.ascontiguousarray(r.transpose(0, 3, 2, 1, 4)).astype(BF16)

    def tile_w2(w):
        # [depth, ff, D] -> [depth, MT, 128, ft, 128]
        r = w.reshape(depth, ft, 128, MT, 128)
        return np.ascontiguousarray(r.transpose(0, 3, 2, 1, 4)).astype(BF16)

    def tile_bias(b, nt):
        # [depth, X] -> [depth, 128, X//128] partition-major
        return np.ascontiguousarray(
            b.reshape(depth, nt, 128).transpose(0, 2, 1)).astype(np.float32)

    def bcast_bias(b):
        return np.ascontiguousarray(
            np.broadcast_to(b.reshape(depth, 1, D), (depth, 128, D))
        ).astype(np.float32)

    # masked keys are dead weight: permute unmasked tokens to the front,
    # so the live keys are a contiguous prefix of the gathered k/v
    perms = []
    for b_idx in range(B):
        live = np.nonzero(mask[b_idx, :s] != 0)[0]
        dead = np.nonzero(mask[b_idx, :s] == 0)[0]
        perms.append(np.concatenate([live, dead]))
    n_live = [int((mask[b_idx, :s] != 0).sum()) for b_idx in range(B)]
    kt_eff = max(1, max((u + 127) // 128 for u in n_live))
    keysp = kt_eff * 128

    shared = {
        "wq": tile_w_cols(Wq, 128),
        "wk": tile_w_cols(Wk, 128),
        "wv": tile_w_cols(Wv, 256),
        "w1": tile_w_cols(W1, 128),
        "w2": tile_w2(W2),
        "bq": tile_bias(bq, MT),
        "bk": tile_bias(bk, MT),
        "b1": tile_bias(b1, ft),
        "b2": tile_bias(b2, MT),
        "bv": bcast_bias(bv),
    }
    in_maps = []
    for c in range(n_cores):
        b_idx, r_idx = divmod(c, group)
        xp = x[b_idx][perms[b_idx]]                           # [s, D] permuted
        xl = xp[r_idx * s_loc : (r_idx + 1) * s_loc, :]       # [s_loc, D]
        xT = np.ascontiguousarray(
            xl.T.reshape(MT, 128, tok).transpose(1, 0, 2)).astype(np.float32)
        u = n_live[b_idx]
        mb = np.full(keysp, NEG, np.float32)
        mb[:u] = 0.0
        mb = np.ascontiguousarray(mb.reshape(kt_eff, 128).T)
        in_maps.append({"xT": xT, "maskb": mb, **shared})
    return in_maps, kt_eff, perms


def _assemble(results, perms, s_loc=S_LOC, n_cores=N_CORES, group=GROUP):
    s = group * s_loc
    out = np.empty((B, s, D), dtype=np.float32)
    for c in range(n_cores):
        b_idx, r_idx = divmod(c, group)
        yT = results[c]["yT"]  # [128, MT, tok]
        xl = yT.transpose(1, 0, 2).reshape(D, s_loc).T
        out[b_idx, perms[b_idx][r_idx * s_loc : (r_idx + 1) * s_loc]] = xl
    return out


_NC_CACHE = {}


def run(inputs, trace=False):
    in_maps, kt_eff, perms = _prep_inputs(**inputs)
    if kt_eff not in _NC_CACHE:
        _NC_CACHE[kt_eff] = build_nc(kt_eff=kt_eff)
    nc = _NC_CACHE[kt_eff]
    res = bass_utils.run_bass_kernel_spmd(
        nc, in_maps, core_ids=list(range(N_CORES)), trace=trace)
    return _assemble(res.results, perms), res


def kernel(**inputs):
    inputs = {k: np.asarray(v) for k, v in inputs.items()}
    out, _ = run(inputs)
    return out


# revision 48
# speedup vs baseline: 1.0652x; 1.0194x over previous
"""Trainium2 Bass kernel for a 4-layer dense transformer block (nn_Block_spe).

Sharding: 8 cores = 2 groups of 4 (one group per batch element). Within a
group each core owns 512 query rows (sequence split). Per layer each core
projects q/k/v for its own rows, two AllGathers (bf16, groups of 4) share
k/v across the group, attention + MLP run fully local to the core's rows.
Residual stream stays in fp32 on-chip across all 4 layers; matmuls in bf16.

Masked-key compaction: the host permutes each batch's tokens so unmasked
tokens form a prefix (the block has no positional encoding, so attention
is permutation-equivariant; outputs are un-permuted at the end). Only the
first ceil(unmasked/128) key tiles are loaded and attended over — masked
keys would contribute exactly zero attention weight anyway. Padding slots
in the last tile are killed by a -1e30 bias fused into the exp.

Everything on-chip uses the transposed [feature, token] layout so matmul
lhsT operands are the natural weight layouts and rhs is always [128, 512].
"""

import numpy as np
import ml_dtypes

import concourse.bass as bass
import concourse.mybir as mybir
import concourse.tile as tile
from concourse import bacc, bass_utils
from concourse.masks import make_identity

dt = mybir.dt
AF = mybir.ActivationFunctionType
BF16 = ml_dtypes.bfloat16

B, S, D = 2, 2048, 1024
H, HD = 16, 64
FF = 4096
DEPTH = 4
N_CORES = 8
GROUP = 4
S_LOC = S // GROUP          # 512 tokens per core
TOK = S_LOC                 # free dim of every matmul
MT = D // 128               # 8 feature tiles
KT = S // 128               # 16 key tiles (global)
FT = FF // 128              # 32 ff tiles
TT = S_LOC // 128           # 4 local token tiles
REPLICA_GROUPS = [[0, 1, 2, 3], [4, 5, 6, 7]]
SCALE = float(HD) ** -0.5
NEG = -1e30


def build_nc(s_loc=S_LOC, depth=DEPTH, ff=FF, n_cores=N_CORES, group=GROUP,
             kt_eff=None, sim_safe=False, local_collective=False):
    """Build the per-core SPMD program. Parametrized so a scaled-down
    version can run under CoreSim (sim_safe swaps Gelu for Identity since
    CoreSim lacks a Gelu implementation). local_collective replaces the
    AllGather with equivalent local DMA traffic so the single-core
    TimelineSim cost model can time the kernel.

    kt_eff: number of 128-key tiles after masked-key compaction (the host
    permutes unmasked tokens to the front, so live keys are a contiguous
    prefix of the AllGather output; padding slots carry a -1e30 exp bias)."""
    act_mlp = AF.Identity if sim_safe else AF.Gelu
    tok = s_loc
    tt = s_loc // 128
    kt_n = group * tt
    if kt_eff is None:
        kt_eff = kt_n
    keysp = kt_eff * 128
    ft = ff // 128
    groups = [list(range(g * group, (g + 1) * group)) for g in range(n_cores // group)]

    nc = bacc.Bacc("TRN2", num_devices=n_cores, debug=False)

    # ---- DRAM I/O (host pre-tiles everything; see kernel() below) ----
    xT_d = nc.dram_tensor("xT", [128, MT, tok], dt.float32, kind="ExternalInput")
    mask_d = nc.dram_tensor("maskb", [128, kt_eff], dt.float32, kind="ExternalInput")
    wq_d = nc.dram_tensor("wq", [depth, MT, 128, MT, 128], dt.bfloat16, kind="ExternalInput")
    wk_d = nc.dram_tensor("wk", [depth, MT, 128, MT, 128], dt.bfloat16, kind="ExternalInput")
    wv_d = nc.dram_tensor("wv", [depth, 4, 128, MT, 256], dt.bfloat16, kind="ExternalInput")
    w1_d = nc.dram_tensor("w1", [depth, ft, 128, MT, 128], dt.bfloat16, kind="ExternalInput")
    w2_d = nc.dram_tensor("w2", [depth, MT, 128, ft, 128], dt.bfloat16, kind="ExternalInput")
    bq_d = nc.dram_tensor("bq", [depth, 128, MT], dt.float32, kind="ExternalInput")
    bk_d = nc.dram_tensor("bk", [depth, 128, MT], dt.float32, kind="ExternalInput")
    bv_d = nc.dram_tensor("bv", [depth, 128, D], dt.float32, kind="ExternalInput")
    b1_d = nc.dram_tensor("b1", [depth, 128, ft], dt.float32, kind="ExternalInput")
    b2_d = nc.dram_tensor("b2", [depth, 128, MT], dt.float32, kind="ExternalInput")
    y_d = nc.dram_tensor("yT", [128, MT, tok], dt.float32, kind="ExternalOutput")

    kblk = D * tok            # bf16 elems per rank in the k half of the AG
    vblk = s_loc * D          # bf16 elems per rank in the v half

    with tile.TileContext(nc) as tc:
        with (
            tc.tile_pool(name="per", bufs=1) as per,     # persistent state
            tc.tile_pool(name="sb", bufs=2) as sb,       # staging / per-layer
            tc.tile_pool(name="sb1", bufs=1) as sb1,     # k/v staging (single)
            tc.tile_pool(name="se", bufs=3) as se,       # expT pipeline
            tc.tile_pool(name="wp", bufs=5) as wp,       # small weight chunks
            tc.tile_pool(name="wp2", bufs=3) as wp2,     # big weight chunks
            tc.tile_pool(name="ps", bufs=2, space="PSUM") as ps,
            tc.tile_pool(name="ps2", bufs=2, space="PSUM") as ps2,
            tc.tile_pool(name="po_p", bufs=1, space="PSUM") as po_p,
            tc.tile_pool(name="dram", bufs=2, space="DRAM") as dram,
        ):
            # persistent tiles
            xT = per.tile([128, MT, tok], dt.float32)
            xT_bf = per.tile([128, MT, tok], dt.bfloat16)
            qT = per.tile([128, MT, tok], dt.bfloat16)
            kT_full = per.tile([128, MT, keysp], dt.bfloat16)
            V_int = per.tile([128, kt_eff, H, HD + 1], dt.bfloat16)
            oT = per.tile([128, MT, tok], dt.bfloat16)
            hT = per.tile([128, ft, tok], dt.bfloat16)
            maskb = per.tile([128, kt_eff], dt.float32)
            ident = per.tile([128, 128], dt.bfloat16)
            bv_bc = per.tile([128, D], dt.float32)

            make_identity(nc, ident[:])
            nc.sync.dma_start(xT[:], xT_d.ap())
            nc.sync.dma_start(maskb[:], mask_d.ap())
            nc.gpsimd.memset(V_int[:, :, :, HD], 1.0)

            for l in range(depth):
                # ---- biases for this layer ----
                bq_t = sb.tile([128, MT], dt.float32, tag="bq")
                b1_t = sb.tile([128, ft], dt.float32, tag="b1")
                b2_t = sb.tile([128, MT], dt.float32, tag="b2")
                nc.sync.dma_start(bq_t[:], bq_d.ap()[l])
                nc.sync.dma_start(b1_t[:], b1_d.ap()[l])
                nc.sync.dma_start(b2_t[:], b2_d.ap()[l])
                nc.sync.dma_start(bv_bc[:], bv_d.ap()[l])
                bk_t = sb.tile([128, MT], dt.float32, tag="bk")
                nc.sync.dma_start(bk_t[:], bk_d.ap()[l])

                # ---- downcast residual for matmul inputs (per m-tile so the
                # next layer's projections can overlap this layer's MLP tail)
                for m in range(MT):
                    nc.vector.tensor_copy(xT_bf[:, m, :], xT[:, m, :])

                agin_k = dram.tile([kblk], dt.bfloat16, tag="agin_k")
                agin_v = dram.tile([vblk], dt.bfloat16, tag="agin_v")

                # ---- k projection (transposed feature-major layout) ----
                kst = sb1.tile([128, MT, tok], dt.bfloat16, tag="kstage")
                for m in range(MT):
                    wc = wp.tile([128, MT, 128], dt.bfloat16, tag="wqk")
                    nc.sync.dma_start(wc[:], wk_d.ap()[l, m])
                    pk = ps.tile([128, tok], dt.float32, tag="pp")
                    for k in range(MT):
                        nc.tensor.matmul(pk[:], lhsT=wc[:, k, :], rhs=xT_bf[:, k, :],
                                         start=(k == 0), stop=(k == MT - 1))
                    nc.vector.tensor_scalar_add(kst[:, m, :], pk[:],
                                                bk_t[:, m : m + 1])
                nc.sync.dma_start(
                    agin_k.opt().rearrange("(m p t) -> p m t", p=128, t=tok),
                    kst[:],
                )
                agout_k = dram.tile([group * kblk], dt.bfloat16, tag="agout_k")
                if local_collective:
                    for r in range(group):
                        nc.sync.dma_start(
                            agout_k.opt()[r * kblk : (r + 1) * kblk], agin_k.opt())
                else:
                    nc.gpsimd.collective_compute(
                        "AllGather", mybir.AluOpType.bypass,
                        ins=[agin_k.opt()], outs=[agout_k.opt()],
                        replica_groups=groups,
                    )

                # ---- v projection (natural layout) ----
                vst = sb1.tile([128, tt, D], dt.bfloat16, tag="vstage")
                for cc in range(4):
                    wc = wp2.tile([128, MT, 256], dt.bfloat16, tag="wv")
                    nc.sync.dma_start(wc[:], wv_d.ap()[l, cc])
                    for t in range(tt):
                        pv = ps.tile([128, 256], dt.float32, tag="pp")
                        for k in range(MT):
                            nc.tensor.matmul(
                                pv[:], lhsT=xT_bf[:, k, t * 128 : (t + 1) * 128],
                                rhs=wc[:, k, :],
                                start=(k == 0), stop=(k == MT - 1))
                        nc.vector.tensor_add(
                            vst[:, t, cc * 256 : (cc + 1) * 256], pv[:],
                            bv_bc[:, cc * 256 : (cc + 1) * 256])
                nc.sync.dma_start(
                    agin_v.opt().rearrange("(t p c) -> p t c", p=128, c=D),
                    vst[:],
                )
                agout_v = dram.tile([group * vblk], dt.bfloat16, tag="agout_v")
                if local_collective:
                    for r in range(group):
                        nc.sync.dma_start(
                            agout_v.opt()[r * vblk : (r + 1) * vblk], agin_v.opt())
                else:
                    nc.gpsimd.collective_compute(
                        "AllGather", mybir.AluOpType.bypass,
                        ins=[agin_v.opt()], outs=[agout_v.opt()],
                        replica_groups=groups,
                    )

                # ---- q projection (overlaps the AllGathers) ----
                for m in range(MT):
                    wc = wp.tile([128, MT, 128], dt.bfloat16, tag="wqk")
                    nc.sync.dma_start(wc[:], wq_d.ap()[l, m])
                    pq = ps.tile([128, tok], dt.float32, tag="pp")
                    for k in range(MT):
                        nc.tensor.matmul(pq[:], lhsT=wc[:, k, :], rhs=xT_bf[:, k, :],
                                         start=(k == 0), stop=(k == MT - 1))
                    nc.vector.tensor_scalar_add(qT[:, m, :], pq[:],
                                                bq_t[:, m : m + 1])

                # ---- gather unmasked k/v rows out of the AllGather output ----
                # k rows transposed on the fly into feature-major [128, MT, keysp]
                # tokens are host-permuted so unmasked keys are the prefix:
                # only the first kt_eff tiles of the gathered k/v are loaded
                kt_done = 0
                for r in range(group):
                    cols = min(tok, keysp - r * tok)
                    if cols <= 0:
                        break
                    nc.sync.dma_start(
                        kT_full[:, :, r * tok : r * tok + cols],
                        agout_k.opt()[r * kblk : (r + 1) * kblk].rearrange(
                            "(m p t) -> p m t", p=128, t=tok)[:, :, 0:cols],
                    )
                    for t in range(cols // 128):
                        vofs = r * vblk + t * 128 * D
                        nc.sync.dma_start(
                            V_int[:, kt_done, :, 0:HD],
                            agout_v.opt()[vofs : vofs + 128 * D].rearrange(
                                "(p h d) -> p h d", p=128, h=H),
                        )
                        kt_done += 1

                # ---- attention, two heads interleaved (hides the
                # scores->exp->AV round-trip latency behind the other head's
                # matmuls) ----
                for hp in range(H // 2):
                    po0 = po_p.tile([65, tok], dt.float32, tag="po0")
                    po1 = po_p.tile([65, tok], dt.float32, tag="po1")
                    for kti in range(kt_eff):
                        pse = ps2.tile([128, 2 * tok], dt.float32, tag="pp2")
                        for rem in range(2):
                            nc.tensor.matmul(
                                pse[:, rem * tok : (rem + 1) * tok],
                                lhsT=kT_full[64 * rem : 64 * rem + 64, hp,
                                             kti * 128 : (kti + 1) * 128],
                                rhs=qT[64 * rem : 64 * rem + 64, hp, :],
                                start=True, stop=True)
                        # one double-width exp: both heads of the pair share
                        # the same key-tile mask column
                        et = se.tile([128, 2 * tok], dt.bfloat16, tag="expT")
                        nc.scalar.activation(et[:], pse[:], AF.Exp,
                                             bias=maskb[:, kti : kti + 1],
                                             scale=SCALE)
                        for rem, po in enumerate((po0, po1)):
                            nc.tensor.matmul(po[:], lhsT=V_int[:, kti, 2 * hp + rem, :],
                                             rhs=et[:, rem * tok : (rem + 1) * tok],
                                             start=(kti == 0), stop=(kti == kt_eff - 1))
                    for rem, po in enumerate((po0, po1)):
                        # copy out first: frees the PSUM bank for the next
                        # pair's AV without waiting on the normalize chain
                        po_s = sb.tile([65, tok], dt.float32, tag="po_s")
                        nc.vector.tensor_copy(po_s[:], po[:])
                        rec = sb.tile([1, tok], dt.float32, tag="rec")
                        nc.vector.reciprocal(rec[:], po_s[64:65, :])
                        bc = sb.tile([64, tok], dt.float32, tag="bc")
                        nc.gpsimd.partition_broadcast(bc[:], rec[:])
                        nc.vector.tensor_mul(
                            oT[64 * rem : 64 * rem + 64, hp, :], po_s[0:64, :], bc[:])

                # ---- MLP up (gelu), ff-tile pairs share a 2-bank psum ----
                for f in range(0, ft, 2):
                    ph = ps2.tile([128, 2 * tok], dt.float32, tag="pp2")
                    for fo in range(2):
                        wc = wp.tile([128, MT, 128], dt.bfloat16, tag="w1")
                        nc.sync.dma_start(wc[:], w1_d.ap()[l, f + fo])
                        for k in range(MT):
                            nc.tensor.matmul(ph[:, fo * tok : (fo + 1) * tok],
                                             lhsT=wc[:, k, :], rhs=oT[:, k, :],
                                             start=(k == 0), stop=(k == MT - 1))
                    for fo in range(2):
                        nc.scalar.activation(hT[:, f + fo, :],
                                             ph[:, fo * tok : (fo + 1) * tok],
                                             act_mlp,
                                             bias=b1_t[:, f + fo : f + fo + 1],
                                             scale=1.0)

                # ---- MLP down + o + residual ----
                hft = ft // 2
                for m in range(MT):
                    pm = ps.tile([128, tok], dt.float32, tag="pp")
                    for hc in range(2):
                        wc = wp2.tile([128, hft, 128], dt.bfloat16, tag="w2")
                        nc.sync.dma_start(wc[:], w2_d.ap()[l, m, :, hc * hft : (hc + 1) * hft, :])
                        for k in range(hft):
                            nc.tensor.matmul(pm[:], lhsT=wc[:, k, :],
                                             rhs=hT[:, hc * hft + k, :],
                                             start=(hc == 0 and k == 0), stop=False)
                    nc.tensor.matmul(pm[:], lhsT=ident[:], rhs=oT[:, m, :],
                                     start=False, stop=True)
                    tmp = sb.tile([128, tok], dt.float32, tag="tmp")
                    nc.vector.tensor_scalar_add(tmp[:], pm[:], b2_t[:, m : m + 1])
                    nc.vector.tensor_add(xT[:, m, :], xT[:, m, :], tmp[:])

            nc.sync.dma_start(y_d.ap(), xT[:])

    nc.compile()
    return nc


def _prep_inputs(x, mask, Wq, bq, Wk, bk, Wv, bv, W1, b1, W2, b2,
                 s_loc=S_LOC, depth=DEPTH, ff=FF, n_cores=N_CORES, group=GROUP):
    """Host-side shard + pre-tile. Tokens of each batch are permuted so
    unmasked tokens form a prefix: attention inside the block is
    permutation-equivariant (no positional encoding), so only the prefix
    ever needs to participate as keys. Returns (in_maps, kt_eff, perms)."""
    tok = s_loc
    ft = ff // 128
    s = group * s_loc

    def tile_w_cols(w, ncol):
        # [depth, D, Dout] -> [depth, Dout//ncol, 128, MT, ncol] bf16
        d_out = w.shape[2]
        r = w.reshape(depth, MT, 128, d_out // ncol, ncol)
        return np.ascontiguousarray(r.transpose(0, 3, 2, 1, 4)).astype(BF16)

    def tile_w2(w):
        # [depth, ff, D] -> [depth, MT, 128, ft, 128]
        r = w.reshape(depth, ft, 128, MT, 128)
        return np.ascontiguousarray(r.transpose(0, 3, 2, 1, 4)).astype(BF16)

    def tile_bias(b, nt):
        # [depth, X] -> [depth, 128, X//128] partition-major
        return np.ascontiguousarray(
            b.reshape(depth, nt, 128).transpose(0, 2, 1)).astype(np.float32)

    def bcast_bias(b):
        return np.ascontiguousarray(
            np.broadcast_to(b.reshape(depth, 1, D), (depth, 128, D))
        ).astype(np.float32)

    # masked keys are dead weight: permute unmasked tokens to the front,
    # so the live keys are a contiguous prefix of the gathered k/v
    perms = []
    for b_idx in range(B):
        live = np.nonzero(mask[b_idx, :s] != 0)[0]
        dead = np.nonzero(mask[b_idx, :s] == 0)[0]
        perms.append(np.concatenate([live, dead]))
    n_live = [int((mask[b_idx, :s] != 0).sum()) for b_idx in range(B)]
    kt_eff = max(1, max((u + 127) // 128 for u in n_live))
    keysp = kt_eff * 128

    shared = {
        "wq": tile_w_cols(Wq, 128),
        "wk": tile_w_cols(Wk, 128),
        "wv": tile_w_cols(Wv, 256),
        "w1": tile_w_cols(W1, 128),
        "w2": tile_w2(W2),
        "bq": tile_bias(bq, MT),
        "bk": tile_bias(bk, MT),
        "b1": tile_bias(b1, ft),
        "b2": tile_bias(b2, MT),
        "bv": bcast_bias(bv),
    }
    in_maps = []
    for c in range(n_cores):
        b_idx, r_idx = divmod(c, group)
        xp = x[b_idx][perms[b_idx]]                           # [s, D] permuted
        xl = xp[r_idx * s_loc : (r_idx + 1) * s_loc, :]       # [s_loc, D]
        xT = np.ascontiguousarray(
            xl.T.reshape(MT, 128, tok).transpose(1, 0, 2)).astype(np.float32)
        u = n_live[b_idx]
        mb = np.full(keysp, NEG, np.float32)
        mb[:u] = 0.0
        mb = np.ascontiguousarray(mb.reshape(kt_eff, 128).T)
        in_maps.append({"xT": xT, "maskb": mb, **shared})
    return in_maps, kt_eff, perms


def _assemble(results, perms, s_loc=S_LOC, n_cores=N_CORES, group=GROUP):
    s = group * s_loc
    out = np.empty((B, s, D), dtype=np.float32)
    for c in range(n_cores):
        b_idx, r_idx = divmod(c, group)
        yT = results[c]["yT"]  # [128, MT, tok]
        xl = yT.transpose(1, 0, 2).reshape(D, s_loc).T
        out[b_idx, perms[b_idx][r_idx * s_loc : (r_idx + 1) * s_loc]] = xl
    return out


_NC_CACHE = {}


def run(inputs, trace=False):
    in_maps, kt_eff, perms = _prep_inputs(**inputs)
    if kt_eff not in _NC_CACHE:
        _NC_CACHE[kt_eff] = build_nc(kt_eff=kt_eff)
    nc = _NC_CACHE[kt_eff]
    res = bass_utils.run_bass_kernel_spmd(
        nc, in_maps, core_ids=list(range(N_CORES)), trace=trace)
    return _assemble(res.results, perms), res


def kernel(**inputs):
    inputs = {k: np.asarray(v) for k, v in inputs.items()}
    out, _ = run(inputs)
    return out


# revision 49
# speedup vs baseline: 1.0700x; 1.0046x over previous
"""Trainium2 Bass kernel for a 4-layer dense transformer block (nn_Block_spe).

Sharding: 8 cores = 2 groups of 4 (one group per batch element). Within a
group each core owns 512 query rows (sequence split). Per layer each core
projects q/k/v for its own rows, two AllGathers (bf16, groups of 4) share
k/v across the group, attention + MLP run fully local to the core's rows.
Residual stream stays in fp32 on-chip across all 4 layers; matmuls in bf16.

Masked-key compaction: the host permutes each batch's tokens so unmasked
tokens form a prefix (the block has no positional encoding, so attention
is permutation-equivariant; outputs are un-permuted at the end). Only the
first ceil(unmasked/128) key tiles are loaded and attended over — masked
keys would contribute exactly zero attention weight anyway. Padding slots
in the last tile are killed by a -1e30 bias fused into the exp.

Everything on-chip uses the transposed [feature, token] layout so matmul
lhsT operands are the natural weight layouts and rhs is always [128, 512].
"""

import numpy as np
import ml_dtypes

import concourse.bass as bass
import concourse.mybir as mybir
import concourse.tile as tile
from concourse import bacc, bass_utils
from concourse.masks import make_identity

dt = mybir.dt
AF = mybir.ActivationFunctionType
BF16 = ml_dtypes.bfloat16

B, S, D = 2, 2048, 1024
H, HD = 16, 64
FF = 4096
DEPTH = 4
N_CORES = 8
GROUP = 4
S_LOC = S // GROUP          # 512 tokens per core
TOK = S_LOC                 # free dim of every matmul
MT = D // 128               # 8 feature tiles
KT = S // 128               # 16 key tiles (global)
FT = FF // 128              # 32 ff tiles
TT = S_LOC // 128           # 4 local token tiles
REPLICA_GROUPS = [[0, 1, 2, 3], [4, 5, 6, 7]]
SCALE = float(HD) ** -0.5
NEG = -1e30


def build_nc(s_loc=S_LOC, depth=DEPTH, ff=FF, n_cores=N_CORES, group=GROUP,
             kt_eff=None, sim_safe=False, local_collective=False):
    """Build the per-core SPMD program. Parametrized so a scaled-down
    version can run under CoreSim (sim_safe swaps Gelu for Identity since
    CoreSim lacks a Gelu implementation). local_collective replaces the
    AllGather with equivalent local DMA traffic so the single-core
    TimelineSim cost model can time the kernel.

    kt_eff: number of 128-key tiles after masked-key compaction (the host
    permutes unmasked tokens to the front, so live keys are a contiguous
    prefix of the AllGather output; padding slots carry a -1e30 exp bias)."""
    act_mlp = AF.Identity if sim_safe else AF.Gelu
    tok = s_loc
    tt = s_loc // 128
    kt_n = group * tt
    if kt_eff is None:
        kt_eff = kt_n
    keysp = kt_eff * 128
    ft = ff // 128
    groups = [list(range(g * group, (g + 1) * group)) for g in range(n_cores // group)]

    nc = bacc.Bacc("TRN2", num_devices=n_cores, debug=False)

    # ---- DRAM I/O (host pre-tiles everything; see kernel() below) ----
    xT_d = nc.dram_tensor("xT", [128, MT, tok], dt.float32, kind="ExternalInput")
    mask_d = nc.dram_tensor("maskb", [128, kt_eff], dt.float32, kind="ExternalInput")
    wq_d = nc.dram_tensor("wq", [depth, MT, 128, MT, 128], dt.bfloat16, kind="ExternalInput")
    wk_d = nc.dram_tensor("wk", [depth, MT, 128, MT, 128], dt.bfloat16, kind="ExternalInput")
    wv_d = nc.dram_tensor("wv", [depth, 4, 128, MT, 256], dt.bfloat16, kind="ExternalInput")
    w1_d = nc.dram_tensor("w1", [depth, ft, 128, MT, 128], dt.bfloat16, kind="ExternalInput")
    w2_d = nc.dram_tensor("w2", [depth, MT, 128, ft, 128], dt.bfloat16, kind="ExternalInput")
    bq_d = nc.dram_tensor("bq", [depth, 128, MT], dt.float32, kind="ExternalInput")
    bk_d = nc.dram_tensor("bk", [depth, 128, MT], dt.float32, kind="ExternalInput")
    bv_d = nc.dram_tensor("bv", [depth, 128, D], dt.float32, kind="ExternalInput")
    b1_d = nc.dram_tensor("b1", [depth, 128, ft], dt.float32, kind="ExternalInput")
    b2_d = nc.dram_tensor("b2", [depth, 128, MT], dt.float32, kind="ExternalInput")
    y_d = nc.dram_tensor("yT", [128, MT, tok], dt.float32, kind="ExternalOutput")

    kblk = D * tok            # bf16 elems per rank in the k half of the AG
    vblk = s_loc * D          # bf16 elems per rank in the v half

    with tile.TileContext(nc) as tc:
        with (
            tc.tile_pool(name="per", bufs=1) as per,     # persistent state
            tc.tile_pool(name="sb", bufs=2) as sb,       # staging / per-layer
            tc.tile_pool(name="sb1", bufs=1) as sb1,     # k/v staging (single)
            tc.tile_pool(name="se", bufs=3) as se,       # expT pipeline
            tc.tile_pool(name="wp", bufs=5) as wp,       # small weight chunks
            tc.tile_pool(name="wp2", bufs=3) as wp2,     # big weight chunks
            tc.tile_pool(name="ps", bufs=2, space="PSUM") as ps,
            tc.tile_pool(name="ps2", bufs=2, space="PSUM") as ps2,
            tc.tile_pool(name="po_p", bufs=1, space="PSUM") as po_p,
            tc.tile_pool(name="dram", bufs=2, space="DRAM") as dram,
        ):
            # persistent tiles
            xT = per.tile([128, MT, tok], dt.float32)
            xT_bf = per.tile([128, MT, tok], dt.bfloat16)
            qT = per.tile([128, MT, tok], dt.bfloat16)
            kT_full = per.tile([128, MT, keysp], dt.bfloat16)
            V_int = per.tile([128, kt_eff, H, HD + 1], dt.bfloat16)
            oT = per.tile([128, MT, tok], dt.bfloat16)
            hT = per.tile([128, ft, tok], dt.bfloat16)
            maskb = per.tile([128, kt_eff], dt.float32)
            ident = per.tile([128, 128], dt.bfloat16)
            bv_bc = per.tile([128, D], dt.float32)

            make_identity(nc, ident[:])
            nc.sync.dma_start(xT[:], xT_d.ap())
            nc.sync.dma_start(maskb[:], mask_d.ap())
            nc.gpsimd.memset(V_int[:, :, :, HD], 1.0)

            for l in range(depth):
                # ---- biases for this layer ----
                bq_t = sb.tile([128, MT], dt.float32, tag="bq")
                b1_t = sb.tile([128, ft], dt.float32, tag="b1")
                b2_t = sb.tile([128, MT], dt.float32, tag="b2")
                nc.sync.dma_start(bq_t[:], bq_d.ap()[l])
                nc.sync.dma_start(b1_t[:], b1_d.ap()[l])
                nc.sync.dma_start(b2_t[:], b2_d.ap()[l])
                nc.sync.dma_start(bv_bc[:], bv_d.ap()[l])
                bk_t = sb.tile([128, MT], dt.float32, tag="bk")
                nc.sync.dma_start(bk_t[:], bk_d.ap()[l])

                # ---- downcast residual for matmul inputs (per m-tile so the
                # next layer's projections can overlap this layer's MLP tail)
                for m in range(MT):
                    nc.vector.tensor_copy(xT_bf[:, m, :], xT[:, m, :])

                agin_k = dram.tile([kblk], dt.bfloat16, tag="agin_k")
                agin_v = dram.tile([vblk], dt.bfloat16, tag="agin_v")

                # ---- k projection (transposed feature-major layout) ----
                kst = sb1.tile([128, MT, tok], dt.bfloat16, tag="kstage")
                for m in range(MT):
                    wc = wp.tile([128, MT, 128], dt.bfloat16, tag="wqk")
                    nc.sync.dma_start(wc[:], wk_d.ap()[l, m])
                    pk = ps.tile([128, tok], dt.float32, tag="pp")
                    for k in range(MT):
                        nc.tensor.matmul(pk[:], lhsT=wc[:, k, :], rhs=xT_bf[:, k, :],
                                         start=(k == 0), stop=(k == MT - 1))
                    nc.vector.tensor_scalar_add(kst[:, m, :], pk[:],
                                                bk_t[:, m : m + 1])
                nc.sync.dma_start(
                    agin_k.opt().rearrange("(m p t) -> p m t", p=128, t=tok),
                    kst[:],
                )
                agout_k = dram.tile([group * kblk], dt.bfloat16, tag="agout_k")
                if local_collective:
                    for r in range(group):
                        nc.sync.dma_start(
                            agout_k.opt()[r * kblk : (r + 1) * kblk], agin_k.opt())
                else:
                    nc.gpsimd.collective_compute(
                        "AllGather", mybir.AluOpType.bypass,
                        ins=[agin_k.opt()], outs=[agout_k.opt()],
                        replica_groups=groups,
                    )

                # ---- v projection (natural layout) ----
                vst = sb1.tile([128, tt, D], dt.bfloat16, tag="vstage")
                for cc in range(4):
                    wc = wp2.tile([128, MT, 256], dt.bfloat16, tag="wv")
                    nc.sync.dma_start(wc[:], wv_d.ap()[l, cc])
                    for t in range(tt):
                        pv = ps.tile([128, 256], dt.float32, tag="pp")
                        for k in range(MT):
                            nc.tensor.matmul(
                                pv[:], lhsT=xT_bf[:, k, t * 128 : (t + 1) * 128],
                                rhs=wc[:, k, :],
                                start=(k == 0), stop=(k == MT - 1))
                        nc.vector.tensor_add(
                            vst[:, t, cc * 256 : (cc + 1) * 256], pv[:],
                            bv_bc[:, cc * 256 : (cc + 1) * 256])
                nc.sync.dma_start(
                    agin_v.opt().rearrange("(t p c) -> p t c", p=128, c=D),
                    vst[:],
                )
                agout_v = dram.tile([group * vblk], dt.bfloat16, tag="agout_v")
                if local_collective:
                    for r in range(group):
                        nc.sync.dma_start(
                            agout_v.opt()[r * vblk : (r + 1) * vblk], agin_v.opt())
                else:
                    nc.gpsimd.collective_compute(
                        "AllGather", mybir.AluOpType.bypass,
                        ins=[agin_v.opt()], outs=[agout_v.opt()],
                        replica_groups=groups,
                    )

                # ---- q projection (overlaps the AllGathers) ----
                for m in range(MT):
                    wc = wp.tile([128, MT, 128], dt.bfloat16, tag="wqk")
                    nc.sync.dma_start(wc[:], wq_d.ap()[l, m])
                    pq = ps.tile([128, tok], dt.float32, tag="pp")
                    for k in range(MT):
                        nc.tensor.matmul(pq[:], lhsT=wc[:, k, :], rhs=xT_bf[:, k, :],
                                         start=(k == 0), stop=(k == MT - 1))
                    nc.vector.tensor_scalar_add(qT[:, m, :], pq[:],
                                                bq_t[:, m : m + 1])

                # ---- gather unmasked k/v rows out of the AllGather output ----
                # k rows transposed on the fly into feature-major [128, MT, keysp]
                # tokens are host-permuted so unmasked keys are the prefix:
                # only the first kt_eff tiles of the gathered k/v are loaded
                kt_done = 0
                for r in range(group):
                    cols = min(tok, keysp - r * tok)
                    if cols <= 0:
                        break
                    nc.sync.dma_start(
                        kT_full[:, :, r * tok : r * tok + cols],
                        agout_k.opt()[r * kblk : (r + 1) * kblk].rearrange(
                            "(m p t) -> p m t", p=128, t=tok)[:, :, 0:cols],
                    )
                    for t in range(cols // 128):
                        vofs = r * vblk + t * 128 * D
                        nc.sync.dma_start(
                            V_int[:, kt_done, :, 0:HD],
                            agout_v.opt()[vofs : vofs + 128 * D].rearrange(
                                "(p h d) -> p h d", p=128, h=H),
                        )
                        kt_done += 1

                # ---- attention, two heads interleaved (hides the
                # scores->exp->AV round-trip latency behind the other head's
                # matmuls) ----
                for hp in range(H // 2):
                    po0 = po_p.tile([65, tok], dt.float32, tag="po0")
                    po1 = po_p.tile([65, tok], dt.float32, tag="po1")
                    for kti in range(kt_eff):
                        pse = ps2.tile([128, 2 * tok], dt.float32, tag="pp2")
                        for rem in range(2):
                            nc.tensor.matmul(
                                pse[:, rem * tok : (rem + 1) * tok],
                                lhsT=kT_full[64 * rem : 64 * rem + 64, hp,
                                             kti * 128 : (kti + 1) * 128],
                                rhs=qT[64 * rem : 64 * rem + 64, hp, :],
                                start=True, stop=True)
                        # one double-width exp: both heads of the pair share
                        # the same key-tile mask column
                        et = se.tile([128, 2 * tok], dt.bfloat16, tag="expT")
                        nc.scalar.activation(et[:], pse[:], AF.Exp,
                                             bias=maskb[:, kti : kti + 1],
                                             scale=SCALE)
                        for rem, po in enumerate((po0, po1)):
                            nc.tensor.matmul(po[:], lhsT=V_int[:, kti, 2 * hp + rem, :],
                                             rhs=et[:, rem * tok : (rem + 1) * tok],
                                             start=(kti == 0), stop=(kti == kt_eff - 1))
                    for rem, po in enumerate((po0, po1)):
                        # copy out first: frees the PSUM bank for the next
                        # pair's AV without waiting on the normalize chain
                        po_s = sb.tile([65, tok], dt.float32, tag="po_s")
                        nc.vector.tensor_copy(po_s[:], po[:])
                        rec = sb.tile([1, tok], dt.float32, tag="rec")
                        nc.vector.reciprocal(rec[:], po_s[64:65, :])
                        bc = sb.tile([64, tok], dt.float32, tag="bc")
                        nc.gpsimd.partition_broadcast(bc[:], rec[:])
                        nc.vector.tensor_mul(
                            oT[64 * rem : 64 * rem + 64, hp, :], po_s[0:64, :], bc[:])

                # ---- MLP up (gelu), ff-tile pairs share a 2-bank psum ----
                for f in range(0, ft, 2):
                    ph = ps2.tile([128, 2 * tok], dt.float32, tag="pp2")
                    for fo in range(2):
                        wc = wp.tile([128, MT, 128], dt.bfloat16, tag="w1")
                        nc.sync.dma_start(wc[:], w1_d.ap()[l, f + fo])
                        for k in range(MT):
                            nc.tensor.matmul(ph[:, fo * tok : (fo + 1) * tok],
                                             lhsT=wc[:, k, :], rhs=oT[:, k, :],
                                             start=(k == 0), stop=(k == MT - 1))
                    for fo in range(2):
                        nc.scalar.activation(hT[:, f + fo, :],
                                             ph[:, fo * tok : (fo + 1) * tok],
                                             act_mlp,
                                             bias=b1_t[:, f + fo : f + fo + 1],
                                             scale=1.0)

                # ---- MLP down + o + residual ----
                hft = ft // 2
                for m in range(MT):
                    pm = ps.tile([128, tok], dt.float32, tag="pp")
                    for hc in range(2):
                        wc = wp2.tile([128, hft, 128], dt.bfloat16, tag="w2")
                        nc.sync.dma_start(wc[:], w2_d.ap()[l, m, :, hc * hft : (hc + 1) * hft, :])
                        for k in range(hft):
                            nc.tensor.matmul(pm[:], lhsT=wc[:, k, :],
                                             rhs=hT[:, hc * hft + k, :],
                                             start=(hc == 0 and k == 0),
                                             stop=(hc == 1 and k == hft - 1))
                    tmp = sb.tile([128, tok], dt.float32, tag="tmp")
                    nc.vector.tensor_scalar_add(tmp[:], pm[:], b2_t[:, m : m + 1])
                    nc.vector.tensor_add(tmp[:], tmp[:], oT[:, m, :])
                    nc.vector.tensor_add(xT[:, m, :], xT[:, m, :], tmp[:])

            nc.sync.dma_start(y_d.ap(), xT[:])

    nc.compile()
    return nc


def _prep_inputs(x, mask, Wq, bq, Wk, bk, Wv, bv, W1, b1, W2, b2,
                 s_loc=S_LOC, depth=DEPTH, ff=FF, n_cores=N_CORES, group=GROUP):
    """Host-side shard + pre-tile. Tokens of each batch are permuted so
    unmasked tokens form a prefix: attention inside the block is
    permutation-equivariant (no positional encoding), so only the prefix
    ever needs to participate as keys. Returns (in_maps, kt_eff, perms)."""
    tok = s_loc
    ft = ff // 128
    s = group * s_loc

    def tile_w_cols(w, ncol):
        # [depth, D, Dout] -> [depth, Dout//ncol, 128, MT, ncol] bf16
        d_out = w.shape[2]
        r = w.reshape(depth, MT, 128, d_out // ncol, ncol)
        return np.ascontiguousarray(r.transpose(0, 3, 2, 1, 4)).astype(BF16)

    def tile_w2(w):
        # [depth, ff, D] -> [depth, MT, 128, ft, 128]
        r = w.reshape(depth, ft, 128, MT, 128)
        return np.ascontiguousarray(r.transpose(0, 3, 2, 1, 4)).astype(BF16)

    def tile_bias(b, nt):
        # [depth, X] -> [depth, 128, X//128] partition-major
        return np.ascontiguousarray(
            b.reshape(depth, nt, 128).transpose(0, 2, 1)).astype(np.float32)

    def bcast_bias(b):
        return np.ascontiguousarray(
            np.broadcast_to(b.reshape(depth, 1, D), (depth, 128, D))
        ).astype(np.float32)

    # masked keys are dead weight: permute unmasked tokens to the front,
    # so the live keys are a contiguous prefix of the gathered k/v
    perms = []
    for b_idx in range(B):
        live = np.nonzero(mask[b_idx, :s] != 0)[0]
        dead = np.nonzero(mask[b_idx, :s] == 0)[0]
        perms.append(np.concatenate([live, dead]))
    n_live = [int((mask[b_idx, :s] != 0).sum()) for b_idx in range(B)]
    kt_eff = max(1, max((u + 127) // 128 for u in n_live))
    keysp = kt_eff * 128

    shared = {
        "wq": tile_w_cols(Wq, 128),
        "wk": tile_w_cols(Wk, 128),
        "wv": tile_w_cols(Wv, 256),
        "w1": tile_w_cols(W1, 128),
        "w2": tile_w2(W2),
        "bq": tile_bias(bq, MT),
        "bk": tile_bias(bk, MT),
        "b1": tile_bias(b1, ft),
        "b2": tile_bias(b2, MT),
        "bv": bcast_bias(bv),
    }
    in_maps = []
    for c in range(n_cores):
        b_idx, r_idx = divmod(c, group)
        xp = x[b_idx][perms[b_idx]]                           # [s, D] permuted
        xl = xp[r_idx * s_loc : (r_idx + 1) * s_loc, :]       # [s_loc, D]
        xT = np.ascontiguousarray(
            xl.T.reshape(MT, 128, tok).transpose(1, 0, 2)).astype(np.float32)
        u = n_live[b_idx]
        mb = np.full(keysp, NEG, np.float32)
        mb[:u] = 0.0
        mb = np.ascontiguousarray(mb.reshape(kt_eff, 128).T)
        in_maps.append({"xT": xT, "maskb": mb, **shared})
    return in_maps, kt_eff, perms


def _assemble(results, perms, s_loc=S_LOC, n_cores=N_CORES, group=GROUP):
    s = group * s_loc
    out = np.empty((B, s, D), dtype=np.float32)
    for c in range(n_cores):
        b_idx, r_idx = divmod(c, group)
        yT = results[c]["yT"]  # [128, MT, tok]
        xl = yT.transpose(1, 0, 2).reshape(D, s_loc).T
        out[b_idx, perms[b_idx][r_idx * s_loc : (r_idx + 1) * s_loc]] = xl
    return out


_NC_CACHE = {}


def run(inputs, trace=False):
    in_maps, kt_eff, perms = _prep_inputs(**inputs)
    if kt_eff not in _NC_CACHE:
        _NC_CACHE[kt_eff] = build_nc(kt_eff=kt_eff)
    nc = _NC_CACHE[kt_eff]
    res = bass_utils.run_bass_kernel_spmd(
        nc, in_maps, core_ids=list(range(N_CORES)), trace=trace)
    return _assemble(res.results, perms), res


def kernel(**inputs):
    inputs = {k: np.asarray(v) for k, v in inputs.items()}
    out, _ = run(inputs)
    return out
